# revision 25
# baseline (speedup 1.0000x reference)
"""ConvLSTM classifier kernel for Trainium2 (8 NeuronCores, data-parallel).

Math (per core, batch shard BL=2048):
  for t in 0..T-1:
    gates = conv1d(x_t, w_ih) + conv1d(h, w_hh) + bias     # (BL, 20, 64), 'SAME' K=5
    i,f,o = sigmoid; g = tanh
    c = f*c + i*g ; h = o*tanh(c)
  logit = h . fc_w + fc_b ; p = sigmoid(logit)
  out = 1 - prod_c(1-p_c) * (1-sigmoid(baseline))

Implementation (16 x 128-batch groups processed as 8 pairs, 16 steps):
  - batch on SBUF partitions everywhere.
  - x is im2col'd AND padded on host into full 128-col window blocks
    [T, B, 8, 128]: cols 0..59 x-taps, col 124 = 1.0 (bias row), rest 0
    (h-region cols 64..123 zeroed -> conv edge taps come free). One
    contiguous 4MB gpsimd DMA per step loads the whole-step mega-tile
    [128, 16bg x 8w x 128], quad-buffered over t%4 so neither the h-tail
    writes nor the next sweep's transposes wait on it.
  - per pair: one xbar DMA transpose [128b, 2048] -> [128 taps, 16, 128b]
    builds the matmul lhsT tiles (x-taps rows 0..59, h-taps 64..123,
    bias row 124 -> single LDWEIGHTS per window).
  - 16 matmuls (one per 8-wide l-window) vs banded weights [128, 160];
    g-block weights doubled so one Sigmoid pass covers all four gates
    (tanh(g) = 2*sigmoid(2g) - 1).
  - ScalarE per pair: 2 sigmoids (PSUM->fp16 ifog, contiguous out) and a
    deferred tanh(c) for the previous pair.
  - VectorE per pair (ops span both halves, pair dim merged into the
    window AP dim): v = i*s2g, u = 2v - i, fc = f*c, c = u + fc, then
    h = o*tanh(c) written window-expanded straight into the next step's
    mega-tile h-region (1 core TT + 4 tiny halo TTs; edge taps stay zero
    from the host-zeroed load).
"""

import numpy as np

import concourse.bass as bass
import concourse.bacc as bacc
import concourse.tile as tile
import concourse.mybir as mybir
from concourse import bass_utils

dt = mybir.dt
ALU = mybir.AluOpType
ACT = mybir.ActivationFunctionType

TIME = 16
BATCH = 16384
C = 5
L = 64
NCORES = 8
BL = BATCH // NCORES          # 2048 per core
NW = 8                        # l-windows per row (l_seg = 8)
WJ = 12                       # taps per (window, channel): 8 + 4 halo
X_OFF = 0
H_OFF = 64
BIAS_COL = 124
CG = 68                       # channel block in clmaj layout: 2+64+2 guards


def make_wband(w_ih, w_hh, b_ih, b_hh):
    """Banded weight matrix [128, 160] fp16.

    Rows match window-block columns (after transpose these are lhsT
    partitions). Cols: G*40 + ch*8 + lam, G in (i,f,o,g) order.
    """
    refbase = (0, 5, 15, 10)  # i, f, o, g -> reference channel offsets
    wb = np.zeros((128, 160), np.float32)
    for row0, wmat in ((X_OFF, w_ih), (H_OFF, w_hh)):
        for c in range(C):
            for j in range(WJ):
                r = row0 + c * WJ + j
                for G in range(4):
                    scale = 2.0 if G == 3 else 1.0
                    for ch in range(C):
                        for lam in range(NW):
                            k = j - lam
                            if 0 <= k < 5:
                                wb[r, G * 40 + ch * 8 + lam] = (
                                    scale * wmat[refbase[G] + ch, c, k]
                                )
    bias = (np.asarray(b_ih) + np.asarray(b_hh)).astype(np.float32)
    for G in range(4):
        scale = 2.0 if G == 3 else 1.0
        for ch in range(C):
            for lam in range(NW):
                wb[BIAS_COL, G * 40 + ch * 8 + lam] = scale * bias[refbase[G] + ch]
    return wb.astype(np.float16)


def _ap(base, off, dims):
    """Manual AP over the same tensor as `base` (an AP), keeping its
    partition dim, with free dims `dims` at extra element offset `off`."""
    return bass.AP(
        tensor=base.tensor,
        offset=base.offset + off,
        ap=[list(base.ap[0])] + [list(d) for d in dims],
    )


def build_body(tc, out_dram, xs, wband_d, fcw5_d, consts_d, T, nbg):
    nc = tc.nc
    f16, f32 = dt.float16, dt.float32

    from contextlib import ExitStack
    es = ExitStack()
    pers = es.enter_context(tc.tile_pool(name="pers", bufs=1))
    psum_pool = es.enter_context(tc.tile_pool(name="psum", bufs=2, space="PSUM"))
    xht_pool = es.enter_context(tc.tile_pool(name="xht", bufs=3))
    ifog_pool = es.enter_context(tc.tile_pool(name="ifog", bufs=3))
    vfc_pool = es.enter_context(tc.tile_pool(name="vfc", bufs=2))
    tc_pool = es.enter_context(tc.tile_pool(name="tc", bufs=3))
    fin_pool = es.enter_context(tc.tile_pool(name="fin", bufs=2))

    wband = pers.tile([128, 160], f16, tag="wband")
    nc.sync.dma_start(out=wband, in_=wband_d)
    fcw5 = pers.tile([128, C * L], f16, tag="fcw5")
    nc.gpsimd.dma_start(
        out=fcw5,
        in_=bass.AP(tensor=fcw5_d.tensor, offset=fcw5_d.offset,
                    ap=[[0, 128], [1, C * L]]),
    )
    consts = pers.tile([128, 2], f32, tag="consts")
    nc.gpsimd.dma_start(
        out=consts,
        in_=bass.AP(tensor=consts_d.tensor, offset=consts_d.offset,
                    ap=[[0, 128], [1, 2]]),
    )
    fcbneg = consts[:, 0:1]
    negq = consts[:, 1:2]

    # step mega-tiles: [128, nbg * NW * 128] fp16, quad-buffered over t%4
    # (the h-tail TTs write into xh[(t+1)%4], whose zero-filled load must
    # complete a sweep earlier so the DVE never waits on it)
    NXB = 4
    colsz = nbg * NW * 128
    xh = [pers.tile([128, colsz], f16, tag=f"xh{k}", name=f"xh{k}")
          for k in range(NXB)]

    # c state: ping-pong per block pair
    npair = nbg // 2
    cbuf = [[pers.tile([128, 640], f16, tag=f"c{pp}_{pr}", name=f"c{pp}_{pr}")
             for pr in range(npair)] for pp in range(2)]
    for pr in range(npair):
        nc.vector.memset(cbuf[0][pr], 0.0)

    out_acc = pers.tile([128, nbg], f32, tag="out_acc")

    def load_step(t):
        src = xs[t].rearrange("(g p) c -> p g c", p=128)
        nc.gpsimd.dma_start(out=xh[t % NXB][:], in_=src)

    def load_half(t, h):
        # half-step load (2.1MB) issued mid-sweep so transposes behind it
        # in issue order wait only briefly on its completion
        half_rows = (nbg // 2) * 128
        src = xs[t, h * half_rows:(h + 1) * half_rows].rearrange(
            "(g p) c -> p g c", p=128)
        half_cols = (nbg // 2) * NW * 128
        nc.gpsimd.dma_start(
            out=xh[t % NXB][:, h * half_cols:(h + 1) * half_cols], in_=src)

    # prologue: steps 0..2
    load_step(0)
    load_step(1)
    load_step(2)

    ifog_tiles = {}
    pending = None

    def tail(pr, t):
        """Deferred pair tail: tanh(c), then h = o*tanh(c) window-expanded
        straight into the next step's mega-tile h-region (3 TTs: 8 core
        taps + left/right 2-tap halos from the neighbor windows; edge taps
        stay zero from the host-zeroed DMA load)."""
        ifog_f = ifog_tiles.pop(pr)[:]
        cn = cbuf[(t + 1) % 2][pr]
        tch = tc_pool.tile([128, 640], f16, tag="tch")
        nc.scalar.activation(out=tch, in_=cn[:], func=ACT.Tanh)
        base = pr * 2 * NW * 128 + H_OFF
        dst = xh[(t + 1) % NXB][:]
        # core 8 taps: pair dim merges into the window dim (strides align)
        nc.vector.tensor_tensor(
            out=_ap(dst, base + 2, [[128, 2 * NW], [WJ, C], [1, 8]]),
            in0=_ap(ifog_f, 80, [[160, 2 * NW], [8, C], [1, 8]]),
            in1=_ap(tch[:], 0, [[40, 2 * NW], [8, C], [1, 8]]),
            op=ALU.mult,
        )
        for half in range(2):
            hb = base + half * NW * 128
            io = half * NW * 160
            to = half * NW * 40
            nc.vector.tensor_tensor(
                out=_ap(dst, hb + 128, [[128, NW - 1], [WJ, C], [1, 2]]),
                in0=_ap(ifog_f, io + 80 + 6, [[160, NW - 1], [8, C], [1, 2]]),
                in1=_ap(tch[:], to + 6, [[40, NW - 1], [8, C], [1, 2]]),
                op=ALU.mult,
            )
            nc.vector.tensor_tensor(
                out=_ap(dst, hb + 10, [[128, NW - 1], [WJ, C], [1, 2]]),
                in0=_ap(ifog_f, io + 80 + 160, [[160, NW - 1], [8, C], [1, 2]]),
                in1=_ap(tch[:], to + 40, [[40, NW - 1], [8, C], [1, 2]]),
                op=ALU.mult,
            )

    for t in range(T):
        for pr in range(npair):
            xht = xht_pool.tile([128, 2 * NW, 128], f16, tag="xht")
            nc.sync.dma_start(
                out=xht[:],
                in_=xh[t % NXB][:, pr * 2 * NW * 128:(pr + 1) * 2 * NW * 128],
                transpose=True,
            )

            ifog = ifog_pool.tile([128, 2 * NW * 160], f16, tag="ifog")
            if pending is not None:
                tail(*pending)
            pending = (pr, t)

            for half in range(2):
                slot = psum_pool.tile([128, 4 * 512], f32, tag="gates")
                for w in range(NW):
                    col = (w // 2) * 512 + (w % 2) * 160
                    nc.tensor.matmul(
                        slot[:, col:col + 160],
                        lhsT=xht[:, half * NW + w, :],
                        rhs=wband[:],
                        start=True,
                        stop=True,
                    )
                nc.scalar.activation(
                    out=_ap(ifog[:], half * 1280,
                            [[320, 4], [160, 2], [1, 160]]),
                    in_=_ap(slot[:], 0, [[512, 4], [160, 2], [1, 160]]),
                    func=ACT.Sigmoid,
                )

            ifog_f = ifog[:]
            sl_i = _ap(ifog_f, 0, [[160, 2 * NW], [1, 40]])
            sl_f = _ap(ifog_f, 40, [[160, 2 * NW], [1, 40]])
            sl_g = _ap(ifog_f, 120, [[160, 2 * NW], [1, 40]])

            v = vfc_pool.tile([128, 640], f16, tag="v")
            nc.vector.tensor_tensor(out=v, in0=sl_i, in1=sl_g, op=ALU.mult)
            u = vfc_pool.tile([128, 640], f16, tag="u")
            nc.vector.scalar_tensor_tensor(
                out=u, in0=v[:], scalar=2.0, in1=sl_i,
                op0=ALU.mult, op1=ALU.subtract,
            )
            fc = vfc_pool.tile([128, 640], f16, tag="fc")
            co = cbuf[t % 2][pr]
            nc.vector.tensor_tensor(out=fc, in0=sl_f, in1=co[:], op=ALU.mult)
            cn = cbuf[(t + 1) % 2][pr]
            nc.vector.tensor_tensor(out=cn[:], in0=u[:], in1=fc[:], op=ALU.add)

            ifog_tiles[pr] = ifog

            if t + 3 < T:
                pa, pb = npair // 2, (3 * npair) // 4
                if pr == pa:
                    load_half(t + 3, 0)
                    if pb == pa:
                        load_half(t + 3, 1)
                elif pr == pb:
                    load_half(t + 3, 1)

    tail(*pending)

    # --- final FC / combine ---
    hfin = xh[T % NXB][:]
    for bg in range(nbg):
        hview = _ap(hfin, bg * NW * 128 + H_OFF + 2, [[128, NW], [WJ, C], [1, 8]])
        fview = _ap(fcw5[:], 0, [[8, NW], [L, C], [1, 8]])
        tmp5 = fin_pool.tile([128, C * L], f32, tag="tmp5")
        nc.vector.tensor_tensor(
            out=_ap(tmp5[:], 0, [[8, NW], [L, C], [1, 8]]),
            in0=hview, in1=fview, op=ALU.mult)
        nraw = fin_pool.tile([128, C], f32, tag="nraw")
        nc.vector.tensor_reduce(
            out=nraw,
            in_=tmp5[:].rearrange("p (c l) -> p c l", l=L),
            axis=mybir.AxisListType.X,
            op=ALU.add,
        )
        pbar = fin_pool.tile([128, C], f32, tag="pbar")
        nc.scalar.activation(
            out=pbar, in_=nraw[:], func=ACT.Sigmoid, bias=fcbneg, scale=1.0
        )
        q2 = fin_pool.tile([128, 2], f32, tag="q2")
        nc.vector.tensor_tensor(out=q2, in0=pbar[:, 0:2], in1=pbar[:, 2:4],
                                op=ALU.mult)
        prod = fin_pool.tile([128, 1], f32, tag="prod")
        nc.vector.tensor_tensor(out=prod, in0=q2[:, 0:1], in1=q2[:, 1:2],
                                op=ALU.mult)
        nc.vector.tensor_tensor(out=prod, in0=prod[:], in1=pbar[:, 4:5],
                                op=ALU.mult)
        nc.scalar.activation(
            out=out_acc[:, bg:bg + 1], in_=prod[:], func=ACT.Identity,
            bias=1.0, scale=negq
        )
    nc.gpsimd.dma_start(out=out_dram, in_=out_acc[:])
    es.close()


def window_x(x):
    """[T, B, 5, 64] fp32 -> [T, B, NW*128] fp16 padded window blocks.

    Block w cols: 0..59 = x[b, c, w*8 + j - 2] (c*12 + j), col 124 = 1.0,
    everything else 0.
    """
    from numpy.lib.stride_tricks import sliding_window_view
    T, B = x.shape[0], x.shape[1]
    xp = np.pad(x, ((0, 0), (0, 0), (0, 0), (2, 2)))
    win = sliding_window_view(xp, WJ, axis=3)[:, :, :, ::NW, :]  # T,B,C,NW,WJ
    out = np.zeros((T, B, NW, 128), np.float16)
    out[:, :, :, : C * WJ] = (
        win.transpose(0, 1, 3, 2, 4).reshape(T, B, NW, C * WJ)
    )
    out[:, :, :, BIAS_COL] = 1.0
    return out.reshape(T, B, NW * 128)


def host_prep(w_ih, w_hh, b_ih, b_hh, fc_w, fc_b, baseline):
    wband = make_wband(np.asarray(w_ih), np.asarray(w_hh),
                       np.asarray(b_ih), np.asarray(b_hh))
    fcw = np.asarray(fc_w)[0].astype(np.float32)          # (64,)
    fcw5 = np.tile(-fcw, C)[None, :].astype(np.float16)    # (1, 320)
    base = float(np.asarray(baseline)[0])
    sig_base = 1.0 / (1.0 + np.exp(-base))
    consts = np.array([[-float(np.asarray(fc_b)[0]), -(1.0 - sig_base)]],
                      np.float32)
    return wband, fcw5, consts


def build_program(T, nbg):
    nc = bacc.Bacc("TRN2", target_bir_lowering=False, debug=False, num_devices=1)
    xs = nc.dram_tensor("xs", [T, nbg * 128, NW * 128], dt.float16,
                        kind="ExternalInput").ap()
    wband_d = nc.dram_tensor("wband", [128, 160], dt.float16,
                             kind="ExternalInput").ap()
    fcw5_d = nc.dram_tensor("fcw5", [1, C * L], dt.float16,
                            kind="ExternalInput").ap()
    consts_d = nc.dram_tensor("consts", [1, 2], dt.float32,
                              kind="ExternalInput").ap()
    out_d = nc.dram_tensor("out", [128, nbg], dt.float32,
                           kind="ExternalOutput").ap()
    with tile.TileContext(nc) as tc:
        build_body(tc, out_d, xs, wband_d, fcw5_d, consts_d, T, nbg)
    nc.compile()
    return nc


_PROG_CACHE = {}


def kernel(x, w_ih, w_hh, b_ih, b_hh, fc_w, fc_b, baseline):
    x = np.asarray(x)
    T, B = x.shape[0], x.shape[1]
    nbg = (B // NCORES) // 128
    key = (T, nbg)
    if key not in _PROG_CACHE:
        _PROG_CACHE[key] = build_program(T, nbg)
    nc = _PROG_CACHE[key]

    wband, fcw5, consts = host_prep(w_ih, w_hh, b_ih, b_hh, fc_w, fc_b, baseline)
    xw = window_x(x)
    bl = B // NCORES
    in_maps = []
    for core in range(NCORES):
        in_maps.append({
            "xs": np.ascontiguousarray(xw[:, core * bl: (core + 1) * bl]),
            "wband": wband,
            "fcw5": fcw5,
            "consts": consts,
        })
    res = bass_utils.run_bass_kernel_spmd(nc, in_maps, core_ids=list(range(NCORES)))
    out = np.concatenate([r["out"].T.reshape(-1) for r in res.results])
    return out.astype(np.float32)


# revision 26
# speedup vs baseline: 1.1018x; 1.1018x over previous
"""ConvLSTM classifier kernel for Trainium2 (8 NeuronCores, data-parallel).

Math (per core, batch shard BL=2048):
  for t in 0..T-1:
    gates = conv1d(x_t, w_ih) + conv1d(h, w_hh) + bias     # (BL, 20, 64), 'SAME' K=5
    i,f,o = sigmoid; g = tanh
    c = f*c + i*g ; h = o*tanh(c)
  logit = h . fc_w + fc_b ; p = sigmoid(logit)
  out = 1 - prod_c(1-p_c) * (1-sigmoid(baseline))

Implementation (16 x 128-batch groups processed as 8 pairs, 16 steps):
  - batch on SBUF partitions everywhere.
  - x is im2col'd AND padded on host into full 128-col window blocks
    [T, B, 8, 128]: cols 0..59 x-taps, col 124 = 1.0 (bias row), rest 0
    (h-region cols 64..123 zeroed -> conv edge taps come free). One
    contiguous 4MB gpsimd DMA per step loads the whole-step mega-tile
    [128, 16bg x 8w x 128], quad-buffered over t%4 so neither the h-tail
    writes nor the next sweep's transposes wait on it.
  - per pair: one xbar DMA transpose [128b, 2048] -> [128 taps, 16, 128b]
    builds the matmul lhsT tiles (x-taps rows 0..59, h-taps 64..123,
    bias row 124 -> single LDWEIGHTS per window).
  - 16 matmuls (one per 8-wide l-window) vs banded weights [128, 160];
    g-block weights doubled so one Sigmoid pass covers all four gates
    (tanh(g) = 2*sigmoid(2g) - 1).
  - ScalarE per pair: 2 sigmoids (PSUM->fp16 ifog, contiguous out) and a
    deferred tanh(c) for the previous pair.
  - VectorE per pair (ops span both halves, pair dim merged into the
    window AP dim): v = i*s2g, u = 2v - i, fc = f*c, c = u + fc, then
    h = o*tanh(c) written window-expanded straight into the next step's
    mega-tile h-region (1 core TT + 4 tiny halo TTs; edge taps stay zero
    from the host-zeroed load).
"""

import numpy as np

import concourse.bass as bass
import concourse.bacc as bacc
import concourse.tile as tile
import concourse.mybir as mybir
from concourse import bass_utils

dt = mybir.dt
ALU = mybir.AluOpType
ACT = mybir.ActivationFunctionType

TIME = 16
BATCH = 16384
C = 5
L = 64
NCORES = 8
BL = BATCH // NCORES          # 2048 per core
NW = 8                        # l-windows per row (l_seg = 8)
WJ = 12                       # taps per (window, channel): 8 + 4 halo
X_OFF = 0
H_OFF = 64
BIAS_COL = 124
CG = 68                       # channel block in clmaj layout: 2+64+2 guards


def make_wband(w_ih, w_hh, b_ih, b_hh):
    """Banded weight matrix [128, 160] fp16.

    Rows match window-block columns (after transpose these are lhsT
    partitions). Cols: G*40 + ch*8 + lam, G in (i,f,o,g) order.
    """
    refbase = (0, 5, 15, 10)  # i, f, o, g -> reference channel offsets
    wb = np.zeros((128, 160), np.float32)
    for row0, wmat in ((X_OFF, w_ih), (H_OFF, w_hh)):
        for c in range(C):
            for j in range(WJ):
                r = row0 + c * WJ + j
                for G in range(4):
                    scale = 2.0 if G == 3 else 1.0
                    for ch in range(C):
                        for lam in range(NW):
                            k = j - lam
                            if 0 <= k < 5:
                                wb[r, G * 40 + ch * 8 + lam] = (
                                    scale * wmat[refbase[G] + ch, c, k]
                                )
    bias = (np.asarray(b_ih) + np.asarray(b_hh)).astype(np.float32)
    for G in range(4):
        scale = 2.0 if G == 3 else 1.0
        for ch in range(C):
            for lam in range(NW):
                wb[BIAS_COL, G * 40 + ch * 8 + lam] = scale * bias[refbase[G] + ch]
    return wb.astype(np.float16)


def _ap(base, off, dims):
    """Manual AP over the same tensor as `base` (an AP), keeping its
    partition dim, with free dims `dims` at extra element offset `off`."""
    return bass.AP(
        tensor=base.tensor,
        offset=base.offset + off,
        ap=[list(base.ap[0])] + [list(d) for d in dims],
    )


def build_body(tc, out_dram, xs, wband_d, fcw5_d, consts_d, T, nbg):
    nc = tc.nc
    f16, f32 = dt.float16, dt.float32

    from contextlib import ExitStack
    es = ExitStack()
    pers = es.enter_context(tc.tile_pool(name="pers", bufs=1))
    psum_pool = es.enter_context(tc.tile_pool(name="psum", bufs=2, space="PSUM"))
    xht_pool = es.enter_context(tc.tile_pool(name="xht", bufs=3))
    ifog_pool = es.enter_context(tc.tile_pool(name="ifog", bufs=3))
    vfc_pool = es.enter_context(tc.tile_pool(name="vfc", bufs=2))
    tc_pool = es.enter_context(tc.tile_pool(name="tc", bufs=3))
    fin_pool = es.enter_context(tc.tile_pool(name="fin", bufs=2))

    wband = pers.tile([128, 160], f16, tag="wband")
    nc.sync.dma_start(out=wband, in_=wband_d)
    fcw5 = pers.tile([128, C * L], f16, tag="fcw5")
    nc.gpsimd.dma_start(
        out=fcw5,
        in_=bass.AP(tensor=fcw5_d.tensor, offset=fcw5_d.offset,
                    ap=[[0, 128], [1, C * L]]),
    )
    consts = pers.tile([128, 2], f32, tag="consts")
    nc.gpsimd.dma_start(
        out=consts,
        in_=bass.AP(tensor=consts_d.tensor, offset=consts_d.offset,
                    ap=[[0, 128], [1, 2]]),
    )
    fcbneg = consts[:, 0:1]
    negq = consts[:, 1:2]

    # step mega-tiles: [128, nbg * NW * 128] fp16, quad-buffered over t%4
    # (the h-tail TTs write into xh[(t+1)%4], whose zero-filled load must
    # complete a sweep earlier so the DVE never waits on it)
    NXB = 4
    colsz = nbg * NW * 128
    xh = [pers.tile([128, colsz], f16, tag=f"xh{k}", name=f"xh{k}")
          for k in range(NXB)]

    # c state: ping-pong per block pair
    npair = nbg // 2
    cbuf = [[pers.tile([128, 640], f16, tag=f"c{pp}_{pr}", name=f"c{pp}_{pr}")
             for pr in range(npair)] for pp in range(2)]
    for pr in range(npair):
        nc.vector.memset(cbuf[0][pr], 0.0)

    out_acc = pers.tile([128, nbg], f32, tag="out_acc")

    def load_step(t):
        src = xs[t].rearrange("(g p) c -> p g c", p=128)
        nc.gpsimd.dma_start(out=xh[t % NXB][:], in_=src)

    def load_half(t, h):
        # half-step load (2.1MB) issued mid-sweep so transposes behind it
        # in issue order wait only briefly on its completion
        half_rows = (nbg // 2) * 128
        src = xs[t, h * half_rows:(h + 1) * half_rows].rearrange(
            "(g p) c -> p g c", p=128)
        half_cols = (nbg // 2) * NW * 128
        nc.gpsimd.dma_start(
            out=xh[t % NXB][:, h * half_cols:(h + 1) * half_cols], in_=src)

    # prologue: steps 0..2
    load_step(0)
    load_step(1)
    load_step(2)

    ifog_tiles = {}
    pending = None

    def tail(pr, t):
        """Deferred pair tail: tanh(c), then h = o*tanh(c) window-expanded
        straight into the next step's mega-tile h-region (3 TTs: 8 core
        taps + left/right 2-tap halos from the neighbor windows; edge taps
        stay zero from the host-zeroed DMA load)."""
        ifog_f = ifog_tiles.pop(pr)[:]
        cn = cbuf[(t + 1) % 2][pr]
        tch = tc_pool.tile([128, 640], f16, tag="tch")
        nc.scalar.activation(out=tch, in_=cn[:], func=ACT.Tanh)
        base = pr * 2 * NW * 128 + H_OFF
        dst = xh[(t + 1) % NXB][:]
        # core 8 taps: pair dim merges into the window dim (strides align)
        nc.vector.tensor_tensor(
            out=_ap(dst, base + 2, [[128, 2 * NW], [WJ, C], [1, 8]]),
            in0=_ap(ifog_f, 80, [[160, 2 * NW], [8, C], [1, 8]]),
            in1=_ap(tch[:], 0, [[40, 2 * NW], [8, C], [1, 8]]),
            op=ALU.mult,
        )
        for half in range(2):
            hb = base + half * NW * 128
            io = half * NW * 160
            to = half * NW * 40
            nc.vector.tensor_tensor(
                out=_ap(dst, hb + 128, [[128, NW - 1], [WJ, C], [1, 2]]),
                in0=_ap(ifog_f, io + 80 + 6, [[160, NW - 1], [8, C], [1, 2]]),
                in1=_ap(tch[:], to + 6, [[40, NW - 1], [8, C], [1, 2]]),
                op=ALU.mult,
            )
            nc.vector.tensor_tensor(
                out=_ap(dst, hb + 10, [[128, NW - 1], [WJ, C], [1, 2]]),
                in0=_ap(ifog_f, io + 80 + 160, [[160, NW - 1], [8, C], [1, 2]]),
                in1=_ap(tch[:], to + 40, [[40, NW - 1], [8, C], [1, 2]]),
                op=ALU.mult,
            )

    for t in range(T):
        for pr in range(npair):
            xht = xht_pool.tile([128, 2 * NW, 128], f16, tag="xht")
            nc.sync.dma_start(
                out=xht[:],
                in_=xh[t % NXB][:, pr * 2 * NW * 128:(pr + 1) * 2 * NW * 128],
                transpose=True,
            )

            ifog = ifog_pool.tile([128, 2 * NW * 160], f16, tag="ifog")
            if pending is not None:
                tail(*pending)
            pending = (pr, t)

            for half in range(2):
                slot = psum_pool.tile([128, 4 * 512], f32, tag="gates")
                for w in range(NW):
                    col = (w // 2) * 512 + (w % 2) * 160
                    nc.tensor.matmul(
                        slot[:, col:col + 160],
                        lhsT=xht[:, half * NW + w, :],
                        rhs=wband[:],
                        start=True,
                        stop=True,
                    )
                nc.scalar.activation(
                    out=_ap(ifog[:], half * 1280,
                            [[320, 4], [160, 2], [1, 160]]),
                    in_=_ap(slot[:], 0, [[512, 4], [160, 2], [1, 160]]),
                    func=ACT.Sigmoid,
                )

            ifog_f = ifog[:]
            sl_i = _ap(ifog_f, 0, [[160, 2 * NW], [1, 40]])
            sl_f = _ap(ifog_f, 40, [[160, 2 * NW], [1, 40]])
            sl_g = _ap(ifog_f, 120, [[160, 2 * NW], [1, 40]])

            v = vfc_pool.tile([128, 640], f16, tag="v")
            nc.vector.tensor_tensor(out=v, in0=sl_i, in1=sl_g, op=ALU.mult)
            u = vfc_pool.tile([128, 640], f16, tag="u")
            nc.vector.scalar_tensor_tensor(
                out=u, in0=v[:], scalar=2.0, in1=sl_i,
                op0=ALU.mult, op1=ALU.subtract,
            )
            fc = vfc_pool.tile([128, 640], f16, tag="fc")
            co = cbuf[t % 2][pr]
            nc.vector.tensor_tensor(out=fc, in0=sl_f, in1=co[:], op=ALU.mult)
            cn = cbuf[(t + 1) % 2][pr]
            nc.vector.tensor_tensor(out=cn[:], in0=u[:], in1=fc[:], op=ALU.add)

            ifog_tiles[pr] = ifog

        if t + 3 < T:
            load_step(t + 3)

    tail(*pending)

    # --- final FC / combine ---
    hfin = xh[T % NXB][:]
    for bg in range(nbg):
        hview = _ap(hfin, bg * NW * 128 + H_OFF + 2, [[128, NW], [WJ, C], [1, 8]])
        fview = _ap(fcw5[:], 0, [[8, NW], [L, C], [1, 8]])
        tmp5 = fin_pool.tile([128, C * L], f32, tag="tmp5")
        nc.vector.tensor_tensor(
            out=_ap(tmp5[:], 0, [[8, NW], [L, C], [1, 8]]),
            in0=hview, in1=fview, op=ALU.mult)
        nraw = fin_pool.tile([128, C], f32, tag="nraw")
        nc.vector.tensor_reduce(
            out=nraw,
            in_=tmp5[:].rearrange("p (c l) -> p c l", l=L),
            axis=mybir.AxisListType.X,
            op=ALU.add,
        )
        pbar = fin_pool.tile([128, C], f32, tag="pbar")
        nc.scalar.activation(
            out=pbar, in_=nraw[:], func=ACT.Sigmoid, bias=fcbneg, scale=1.0
        )
        q2 = fin_pool.tile([128, 2], f32, tag="q2")
        nc.vector.tensor_tensor(out=q2, in0=pbar[:, 0:2], in1=pbar[:, 2:4],
                                op=ALU.mult)
        prod = fin_pool.tile([128, 1], f32, tag="prod")
        nc.vector.tensor_tensor(out=prod, in0=q2[:, 0:1], in1=q2[:, 1:2],
                                op=ALU.mult)
        nc.vector.tensor_tensor(out=prod, in0=prod[:], in1=pbar[:, 4:5],
                                op=ALU.mult)
        nc.scalar.activation(
            out=out_acc[:, bg:bg + 1], in_=prod[:], func=ACT.Identity,
            bias=1.0, scale=negq
        )
    nc.gpsimd.dma_start(out=out_dram, in_=out_acc[:])
    es.close()


def window_x(x):
    """[T, B, 5, 64] fp32 -> [T, B, NW*128] fp16 padded window blocks.

    Block w cols: 0..59 = x[b, c, w*8 + j - 2] (c*12 + j), col 124 = 1.0,
    everything else 0.
    """
    from numpy.lib.stride_tricks import sliding_window_view
    T, B = x.shape[0], x.shape[1]
    xp = np.pad(x, ((0, 0), (0, 0), (0, 0), (2, 2)))
    win = sliding_window_view(xp, WJ, axis=3)[:, :, :, ::NW, :]  # T,B,C,NW,WJ
    out = np.zeros((T, B, NW, 128), np.float16)
    out[:, :, :, : C * WJ] = (
        win.transpose(0, 1, 3, 2, 4).reshape(T, B, NW, C * WJ)
    )
    out[:, :, :, BIAS_COL] = 1.0
    return out.reshape(T, B, NW * 128)


def host_prep(w_ih, w_hh, b_ih, b_hh, fc_w, fc_b, baseline):
    wband = make_wband(np.asarray(w_ih), np.asarray(w_hh),
                       np.asarray(b_ih), np.asarray(b_hh))
    fcw = np.asarray(fc_w)[0].astype(np.float32)          # (64,)
    fcw5 = np.tile(-fcw, C)[None, :].astype(np.float16)    # (1, 320)
    base = float(np.asarray(baseline)[0])
    sig_base = 1.0 / (1.0 + np.exp(-base))
    consts = np.array([[-float(np.asarray(fc_b)[0]), -(1.0 - sig_base)]],
                      np.float32)
    return wband, fcw5, consts


def build_program(T, nbg):
    nc = bacc.Bacc("TRN2", target_bir_lowering=False, debug=False, num_devices=1)
    xs = nc.dram_tensor("xs", [T, nbg * 128, NW * 128], dt.float16,
                        kind="ExternalInput").ap()
    wband_d = nc.dram_tensor("wband", [128, 160], dt.float16,
                             kind="ExternalInput").ap()
    fcw5_d = nc.dram_tensor("fcw5", [1, C * L], dt.float16,
                            kind="ExternalInput").ap()
    consts_d = nc.dram_tensor("consts", [1, 2], dt.float32,
                              kind="ExternalInput").ap()
    out_d = nc.dram_tensor("out", [128, nbg], dt.float32,
                           kind="ExternalOutput").ap()
    with tile.TileContext(nc) as tc:
        build_body(tc, out_d, xs, wband_d, fcw5_d, consts_d, T, nbg)
    nc.compile()
    return nc


_PROG_CACHE = {}


def kernel(x, w_ih, w_hh, b_ih, b_hh, fc_w, fc_b, baseline):
    x = np.asarray(x)
    T, B = x.shape[0], x.shape[1]
    nbg = (B // NCORES) // 128
    key = (T, nbg)
    if key not in _PROG_CACHE:
        _PROG_CACHE[key] = build_program(T, nbg)
    nc = _PROG_CACHE[key]

    wband, fcw5, consts = host_prep(w_ih, w_hh, b_ih, b_hh, fc_w, fc_b, baseline)
    xw = window_x(x)
    bl = B // NCORES
    in_maps = []
    for core in range(NCORES):
        in_maps.append({
            "xs": np.ascontiguousarray(xw[:, core * bl: (core + 1) * bl]),
            "wband": wband,
            "fcw5": fcw5,
            "consts": consts,
        })
    res = bass_utils.run_bass_kernel_spmd(nc, in_maps, core_ids=list(range(NCORES)))
    out = np.concatenate([r["out"].T.reshape(-1) for r in res.results])
    return out.astype(np.float32)


# revision 27
# speedup vs baseline: 1.1354x; 1.0305x over previous
"""ConvLSTM classifier kernel for Trainium2 (8 NeuronCores, data-parallel).

Math (per core, batch shard BL=2048):
  for t in 0..T-1:
    gates = conv1d(x_t, w_ih) + conv1d(h, w_hh) + bias     # (BL, 20, 64), 'SAME' K=5
    i,f,o = sigmoid; g = tanh
    c = f*c + i*g ; h = o*tanh(c)
  logit = h . fc_w + fc_b ; p = sigmoid(logit)
  out = 1 - prod_c(1-p_c) * (1-sigmoid(baseline))

Implementation (16 x 128-batch groups processed as 8 pairs, 16 steps):
  - batch on SBUF partitions everywhere.
  - x is im2col'd AND padded on host into full 128-col window blocks
    [T, B, 8, 128]: cols 0..59 x-taps, col 124 = 1.0 (bias row), rest 0
    (h-region cols 64..123 zeroed -> conv edge taps come free). One
    contiguous 4MB gpsimd DMA per step loads the whole-step mega-tile
    [128, 16bg x 8w x 128], quad-buffered over t%4 so neither the h-tail
    writes nor the next sweep's transposes wait on it.
  - per pair: one xbar DMA transpose [128b, 2048] -> [128 taps, 16, 128b]
    builds the matmul lhsT tiles (x-taps rows 0..59, h-taps 64..123,
    bias row 124 -> single LDWEIGHTS per window).
  - 16 matmuls (one per 8-wide l-window) vs banded weights [128, 160];
    g-block weights doubled so one Sigmoid pass covers all four gates
    (tanh(g) = 2*sigmoid(2g) - 1).
  - ScalarE per pair: 2 sigmoids (PSUM->fp16 ifog, contiguous out) and a
    deferred tanh(c) for the previous pair.
  - VectorE per pair (ops span both halves, pair dim merged into the
    window AP dim): v = i*s2g, u = 2v - i, fc = f*c, c = u + fc, then
    h = o*tanh(c) written window-expanded straight into the next step's
    mega-tile h-region (1 core TT + 4 tiny halo TTs; edge taps stay zero
    from the host-zeroed load).
"""

import numpy as np

import concourse.bass as bass
import concourse.bacc as bacc
import concourse.tile as tile
import concourse.mybir as mybir
from concourse import bass_utils

dt = mybir.dt
ALU = mybir.AluOpType
ACT = mybir.ActivationFunctionType

TIME = 16
BATCH = 16384
C = 5
L = 64
NCORES = 8
BL = BATCH // NCORES          # 2048 per core
NW = 8                        # l-windows per row (l_seg = 8)
WJ = 12                       # taps per (window, channel): 8 + 4 halo
X_OFF = 0
H_OFF = 64
BIAS_COL = 124
CG = 68                       # channel block in clmaj layout: 2+64+2 guards


def make_wband(w_ih, w_hh, b_ih, b_hh):
    """Banded weight matrix [128, 160] fp16.

    Rows match window-block columns (after transpose these are lhsT
    partitions). Cols: G*40 + ch*8 + lam, G in (i,f,o,g) order.
    """
    refbase = (0, 5, 15, 10)  # i, f, o, g -> reference channel offsets
    wb = np.zeros((128, 160), np.float32)
    for row0, wmat in ((X_OFF, w_ih), (H_OFF, w_hh)):
        for c in range(C):
            for j in range(WJ):
                r = row0 + c * WJ + j
                for G in range(4):
                    scale = 2.0 if G == 3 else 1.0
                    for ch in range(C):
                        for lam in range(NW):
                            k = j - lam
                            if 0 <= k < 5:
                                wb[r, G * 40 + ch * 8 + lam] = (
                                    scale * wmat[refbase[G] + ch, c, k]
                                )
    bias = (np.asarray(b_ih) + np.asarray(b_hh)).astype(np.float32)
    for G in range(4):
        scale = 2.0 if G == 3 else 1.0
        for ch in range(C):
            for lam in range(NW):
                wb[BIAS_COL, G * 40 + ch * 8 + lam] = scale * bias[refbase[G] + ch]
    return wb.astype(np.float16)


def _ap(base, off, dims):
    """Manual AP over the same tensor as `base` (an AP), keeping its
    partition dim, with free dims `dims` at extra element offset `off`."""
    return bass.AP(
        tensor=base.tensor,
        offset=base.offset + off,
        ap=[list(base.ap[0])] + [list(d) for d in dims],
    )


def build_body(tc, out_dram, xs, wband_d, fcw5_d, consts_d, T, nbg):
    nc = tc.nc
    f16, f32 = dt.float16, dt.float32

    from contextlib import ExitStack
    es = ExitStack()
    pers = es.enter_context(tc.tile_pool(name="pers", bufs=1))
    psum_pool = es.enter_context(tc.tile_pool(name="psum", bufs=2, space="PSUM"))
    xht_pool = es.enter_context(tc.tile_pool(name="xht", bufs=3))
    ifog_pool = es.enter_context(tc.tile_pool(name="ifog", bufs=3))
    vfc_pool = es.enter_context(tc.tile_pool(name="vfc", bufs=2))
    tc_pool = es.enter_context(tc.tile_pool(name="tc", bufs=3))
    fin_pool = es.enter_context(tc.tile_pool(name="fin", bufs=2))

    wband = pers.tile([128, 160], f16, tag="wband")
    nc.sync.dma_start(out=wband, in_=wband_d)
    fcw5 = pers.tile([128, C * L], f16, tag="fcw5")
    nc.gpsimd.dma_start(
        out=fcw5,
        in_=bass.AP(tensor=fcw5_d.tensor, offset=fcw5_d.offset,
                    ap=[[0, 128], [1, C * L]]),
    )
    consts = pers.tile([128, 2], f32, tag="consts")
    nc.gpsimd.dma_start(
        out=consts,
        in_=bass.AP(tensor=consts_d.tensor, offset=consts_d.offset,
                    ap=[[0, 128], [1, 2]]),
    )
    fcbneg = consts[:, 0:1]
    negq = consts[:, 1:2]

    # step mega-tiles: [128, nbg * NW * 128] fp16, quad-buffered over t%4
    # (the h-tail TTs write into xh[(t+1)%4], whose zero-filled load must
    # complete a sweep earlier so the DVE never waits on it)
    NXB = 4
    colsz = nbg * NW * 128
    xh = [pers.tile([128, colsz], f16, tag=f"xh{k}", name=f"xh{k}")
          for k in range(NXB)]

    # c state: ping-pong per block pair
    npair = nbg // 2
    cbuf = [[pers.tile([128, 640], f16, tag=f"c{pp}_{pr}", name=f"c{pp}_{pr}")
             for pr in range(npair)] for pp in range(2)]
    for pr in range(npair):
        nc.vector.memset(cbuf[0][pr], 0.0)

    out_acc = pers.tile([128, nbg], f32, tag="out_acc")

    def load_step(t):
        # HWDGE via ScalarE: dodges the SWDGE-copy vs xbar-transpose
        # serialization that stalled each sweep boundary ~8us
        src = xs[t].rearrange("(g p) c -> p g c", p=128)
        nc.scalar.dma_start(out=xh[t % NXB][:], in_=src)

    def load_half(t, h):
        # half-step load (2.1MB) issued mid-sweep so transposes behind it
        # in issue order wait only briefly on its completion
        half_rows = (nbg // 2) * 128
        src = xs[t, h * half_rows:(h + 1) * half_rows].rearrange(
            "(g p) c -> p g c", p=128)
        half_cols = (nbg // 2) * NW * 128
        nc.gpsimd.dma_start(
            out=xh[t % NXB][:, h * half_cols:(h + 1) * half_cols], in_=src)

    # prologue: steps 0..2
    load_step(0)
    load_step(1)
    load_step(2)

    ifog_tiles = {}
    pending = None

    def tail(pr, t):
        """Deferred pair tail: tanh(c), then h = o*tanh(c) window-expanded
        straight into the next step's mega-tile h-region (3 TTs: 8 core
        taps + left/right 2-tap halos from the neighbor windows; edge taps
        stay zero from the host-zeroed DMA load)."""
        ifog_f = ifog_tiles.pop(pr)[:]
        cn = cbuf[(t + 1) % 2][pr]
        tch = tc_pool.tile([128, 640], f16, tag="tch")
        nc.scalar.activation(out=tch, in_=cn[:], func=ACT.Tanh)
        base = pr * 2 * NW * 128 + H_OFF
        dst = xh[(t + 1) % NXB][:]
        # core 8 taps: pair dim merges into the window dim (strides align)
        nc.vector.tensor_tensor(
            out=_ap(dst, base + 2, [[128, 2 * NW], [WJ, C], [1, 8]]),
            in0=_ap(ifog_f, 80, [[160, 2 * NW], [8, C], [1, 8]]),
            in1=_ap(tch[:], 0, [[40, 2 * NW], [8, C], [1, 8]]),
            op=ALU.mult,
        )
        for half in range(2):
            hb = base + half * NW * 128
            io = half * NW * 160
            to = half * NW * 40
            nc.vector.tensor_tensor(
                out=_ap(dst, hb + 128, [[128, NW - 1], [WJ, C], [1, 2]]),
                in0=_ap(ifog_f, io + 80 + 6, [[160, NW - 1], [8, C], [1, 2]]),
                in1=_ap(tch[:], to + 6, [[40, NW - 1], [8, C], [1, 2]]),
                op=ALU.mult,
            )
            nc.vector.tensor_tensor(
                out=_ap(dst, hb + 10, [[128, NW - 1], [WJ, C], [1, 2]]),
                in0=_ap(ifog_f, io + 80 + 160, [[160, NW - 1], [8, C], [1, 2]]),
                in1=_ap(tch[:], to + 40, [[40, NW - 1], [8, C], [1, 2]]),
                op=ALU.mult,
            )

    for t in range(T):
        for pr in range(npair):
            xht = xht_pool.tile([128, 2 * NW, 128], f16, tag="xht")
            nc.sync.dma_start(
                out=xht[:],
                in_=xh[t % NXB][:, pr * 2 * NW * 128:(pr + 1) * 2 * NW * 128],
                transpose=True,
            )

            ifog = ifog_pool.tile([128, 2 * NW * 160], f16, tag="ifog")
            if pending is not None:
                tail(*pending)
            pending = (pr, t)

            for half in range(2):
                slot = psum_pool.tile([128, 4 * 512], f32, tag="gates")
                for w in range(NW):
                    col = (w // 2) * 512 + (w % 2) * 160
                    nc.tensor.matmul(
                        slot[:, col:col + 160],
                        lhsT=xht[:, half * NW + w, :],
                        rhs=wband[:],
                        start=True,
                        stop=True,
                    )
                nc.scalar.activation(
                    out=_ap(ifog[:], half * 1280,
                            [[320, 4], [160, 2], [1, 160]]),
                    in_=_ap(slot[:], 0, [[512, 4], [160, 2], [1, 160]]),
                    func=ACT.Sigmoid,
                )

            ifog_f = ifog[:]
            sl_i = _ap(ifog_f, 0, [[160, 2 * NW], [1, 40]])
            sl_f = _ap(ifog_f, 40, [[160, 2 * NW], [1, 40]])
            sl_g = _ap(ifog_f, 120, [[160, 2 * NW], [1, 40]])

            v = vfc_pool.tile([128, 640], f16, tag="v")
            nc.vector.tensor_tensor(out=v, in0=sl_i, in1=sl_g, op=ALU.mult)
            u = vfc_pool.tile([128, 640], f16, tag="u")
            nc.vector.scalar_tensor_tensor(
                out=u, in0=v[:], scalar=2.0, in1=sl_i,
                op0=ALU.mult, op1=ALU.subtract,
            )
            fc = vfc_pool.tile([128, 640], f16, tag="fc")
            co = cbuf[t % 2][pr]
            nc.vector.tensor_tensor(out=fc, in0=sl_f, in1=co[:], op=ALU.mult)
            cn = cbuf[(t + 1) % 2][pr]
            nc.vector.tensor_tensor(out=cn[:], in0=u[:], in1=fc[:], op=ALU.add)

            ifog_tiles[pr] = ifog

        if t + 3 < T:
            load_step(t + 3)

    tail(*pending)

    # --- final FC / combine ---
    hfin = xh[T % NXB][:]
    for bg in range(nbg):
        hview = _ap(hfin, bg * NW * 128 + H_OFF + 2, [[128, NW], [WJ, C], [1, 8]])
        fview = _ap(fcw5[:], 0, [[8, NW], [L, C], [1, 8]])
        tmp5 = fin_pool.tile([128, C * L], f32, tag="tmp5")
        nc.vector.tensor_tensor(
            out=_ap(tmp5[:], 0, [[8, NW], [L, C], [1, 8]]),
            in0=hview, in1=fview, op=ALU.mult)
        nraw = fin_pool.tile([128, C], f32, tag="nraw")
        nc.vector.tensor_reduce(
            out=nraw,
            in_=tmp5[:].rearrange("p (c l) -> p c l", l=L),
            axis=mybir.AxisListType.X,
            op=ALU.add,
        )
        pbar = fin_pool.tile([128, C], f32, tag="pbar")
        nc.scalar.activation(
            out=pbar, in_=nraw[:], func=ACT.Sigmoid, bias=fcbneg, scale=1.0
        )
        q2 = fin_pool.tile([128, 2], f32, tag="q2")
        nc.vector.tensor_tensor(out=q2, in0=pbar[:, 0:2], in1=pbar[:, 2:4],
                                op=ALU.mult)
        prod = fin_pool.tile([128, 1], f32, tag="prod")
        nc.vector.tensor_tensor(out=prod, in0=q2[:, 0:1], in1=q2[:, 1:2],
                                op=ALU.mult)
        nc.vector.tensor_tensor(out=prod, in0=prod[:], in1=pbar[:, 4:5],
                                op=ALU.mult)
        nc.scalar.activation(
            out=out_acc[:, bg:bg + 1], in_=prod[:], func=ACT.Identity,
            bias=1.0, scale=negq
        )
    nc.gpsimd.dma_start(out=out_dram, in_=out_acc[:])
    es.close()


def window_x(x):
    """[T, B, 5, 64] fp32 -> [T, B, NW*128] fp16 padded window blocks.

    Block w cols: 0..59 = x[b, c, w*8 + j - 2] (c*12 + j), col 124 = 1.0,
    everything else 0.
    """
    from numpy.lib.stride_tricks import sliding_window_view
    T, B = x.shape[0], x.shape[1]
    xp = np.pad(x, ((0, 0), (0, 0), (0, 0), (2, 2)))
    win = sliding_window_view(xp, WJ, axis=3)[:, :, :, ::NW, :]  # T,B,C,NW,WJ
    out = np.zeros((T, B, NW, 128), np.float16)
    out[:, :, :, : C * WJ] = (
        win.transpose(0, 1, 3, 2, 4).reshape(T, B, NW, C * WJ)
    )
    out[:, :, :, BIAS_COL] = 1.0
    return out.reshape(T, B, NW * 128)


def host_prep(w_ih, w_hh, b_ih, b_hh, fc_w, fc_b, baseline):
    wband = make_wband(np.asarray(w_ih), np.asarray(w_hh),
                       np.asarray(b_ih), np.asarray(b_hh))
    fcw = np.asarray(fc_w)[0].astype(np.float32)          # (64,)
    fcw5 = np.tile(-fcw, C)[None, :].astype(np.float16)    # (1, 320)
    base = float(np.asarray(baseline)[0])
    sig_base = 1.0 / (1.0 + np.exp(-base))
    consts = np.array([[-float(np.asarray(fc_b)[0]), -(1.0 - sig_base)]],
                      np.float32)
    return wband, fcw5, consts


def build_program(T, nbg):
    nc = bacc.Bacc("TRN2", target_bir_lowering=False, debug=False, num_devices=1)
    xs = nc.dram_tensor("xs", [T, nbg * 128, NW * 128], dt.float16,
                        kind="ExternalInput").ap()
    wband_d = nc.dram_tensor("wband", [128, 160], dt.float16,
                             kind="ExternalInput").ap()
    fcw5_d = nc.dram_tensor("fcw5", [1, C * L], dt.float16,
                            kind="ExternalInput").ap()
    consts_d = nc.dram_tensor("consts", [1, 2], dt.float32,
                              kind="ExternalInput").ap()
    out_d = nc.dram_tensor("out", [128, nbg], dt.float32,
                           kind="ExternalOutput").ap()
    with tile.TileContext(nc) as tc:
        build_body(tc, out_d, xs, wband_d, fcw5_d, consts_d, T, nbg)
    nc.compile()
    return nc


_PROG_CACHE = {}


def kernel(x, w_ih, w_hh, b_ih, b_hh, fc_w, fc_b, baseline):
    x = np.asarray(x)
    T, B = x.shape[0], x.shape[1]
    nbg = (B // NCORES) // 128
    key = (T, nbg)
    if key not in _PROG_CACHE:
        _PROG_CACHE[key] = build_program(T, nbg)
    nc = _PROG_CACHE[key]

    wband, fcw5, consts = host_prep(w_ih, w_hh, b_ih, b_hh, fc_w, fc_b, baseline)
    xw = window_x(x)
    bl = B // NCORES
    in_maps = []
    for core in range(NCORES):
        in_maps.append({
            "xs": np.ascontiguousarray(xw[:, core * bl: (core + 1) * bl]),
            "wband": wband,
            "fcw5": fcw5,
            "consts": consts,
        })
    res = bass_utils.run_bass_kernel_spmd(nc, in_maps, core_ids=list(range(NCORES)))
    out = np.concatenate([r["out"].T.reshape(-1) for r in res.results])
    return out.astype(np.float32)
